# revision 1
# baseline (speedup 1.0000x reference)
"""Trainium2 Bass kernel for BaseGenerator: mapped = mapping @ base_flat.

Strategy (8-core SPMD, pure data-parallel over output pixels):
  - mapping [P1=16384, P0=16384] f32 is row-sharded: core c owns output rows
    [c*2048, (c+1)*2048).  Host pre-transposes each shard to mt_c [P0, 2048]
    (K-major, cast to fp16) so the contraction axis lands on SBUF partitions
    and the device streams the shard with 15 contiguous 4 MiB DMAs plus a
    fine-grained 4x 1 MiB tail (so the final matmuls chase the stream).
  - base_flat [P0, 3] is replicated, rearranged host-side to [128, 128*3] so
    each 128-row K-chunk gives a [128, 3] stationary matmul operand (lhsT).
  - Device: per 8-chunk DMA tile, 32 matmuls (K=128, M=3, N=512) accumulate
    into 4 persistent PSUM banks of [3, 512] f32 across all 128 K-chunks.
    Epilogue copies PSUM -> SBUF -> DRAM out [3, 2048] f32.
  - Host concatenates per-core outputs -> [16384, 3] -> [128, 128, 3].

The kernel is DMA-bound: 64 MiB/core (16-bit) streams at ~406 GB/s/core,
~99% of chip HBM peak across 8 cores; best measured HW exec 189.1 us
(occasional ~217-226 us slow-mode runs are environmental).  COMPUTE_DTYPE
selects the internal precision of the mapping/base operands: "float16"
(shipped) -> absmax rel err 1.1e-5 vs the f32 reference at the same speed
as bf16; "bfloat16" -> 8.8e-5; "float32" -> 1e-6 at ~2.1x the time.
Accumulation is always f32 in PSUM; output is always f32.
"""

import sys

import numpy as np

try:
    import concourse.bacc as bacc
except ImportError:  # fresh env without PYTHONPATH: fall back to repo paths
    for _p in ("/opt/trn_rl_repo", "/opt/pypackages",
               "/root/.axon_site/_ro/trn_rl_repo",
               "/root/.axon_site/_ro/pypackages"):
        if _p not in sys.path:
            sys.path.append(_p)
    import concourse.bacc as bacc
import concourse.bass as bass
import concourse.mybir as mybir
import concourse.tile as tile
from concourse.bass_utils import run_bass_kernel_spmd
from concourse.tile_rust import add_dep_helper

H0 = W0 = 128
H1 = W1 = 128
P0 = H0 * W0          # 16384 contraction length
P1 = H1 * W1          # 16384 output pixels
N_CORES = 8
N_PER_CORE = P1 // N_CORES   # 2048 output pixels per core
KC = 128              # K-chunk size (SBUF partitions)
N_KCHUNKS = P0 // KC  # 128
NB = 512              # matmul moving free dim (one PSUM bank of f32)
N_BANKS = N_PER_CORE // NB   # 4

COMPUTE_DTYPE = "float16"   # fp16: same bytes/speed as bf16, ~8x lower quantization err
CHUNKS_PER_DMA = 8           # K-chunks fetched per dma_start (8 -> 4 MiB bf16)
DMA_BUFS = 4                 # in-flight DMA tiles (5 measured worse)
ALT_DMA_RINGS = False        # alternate sync/scalar HWDGE rings (crashes HW; keep off)
DEDUP_LDWEIGHTS = False      # inert on this stack (backend ignores the flag)

_PROGRAM_CACHE = {}


def _np_compute_dtype(name):
    if name == "float32":
        return np.float32
    if name == "float16":
        return np.float16
    import ml_dtypes
    return ml_dtypes.bfloat16


def _build_program(dtype_name):
    """Build + compile the SPMD Bass program (identical on all 8 cores)."""
    dt = getattr(mybir.dt, dtype_name)
    nc = bacc.Bacc(
        "TRN2", target_bir_lowering=False, debug=False, num_devices=N_CORES
    )
    n_t = N_KCHUNKS // CHUNKS_PER_DMA
    mt = nc.dram_tensor("mt", [n_t * KC, CHUNKS_PER_DMA * N_PER_CORE], dt,
                        kind="ExternalInput")
    bt = nc.dram_tensor("bt", [KC, N_KCHUNKS * 3], dt, kind="ExternalInput")
    out = nc.dram_tensor(
        "out", [3, N_PER_CORE], mybir.dt.float32, kind="ExternalOutput"
    )

    qc = CHUNKS_PER_DMA
    n_dmas = N_KCHUNKS // qc
    # mt[(a*KC)+p, n] viewed as [p, (a n)] so a slice of the free dim covers
    # `qc` consecutive K-chunks in one contiguous-per-partition DMA.
    with tile.TileContext(nc) as tc:
        with (
            tc.tile_pool(name="bpool", bufs=1) as bpool,
            tc.tile_pool(name="mpool", bufs=DMA_BUFS) as mpool,
            tc.tile_pool(name="psum", bufs=1, space=bass.MemorySpace.PSUM) as pp,
            tc.tile_pool(name="opool", bufs=1) as opool,
        ):
            b_sb = bpool.tile([KC, N_KCHUNKS * 3], dt)
            nc.sync.dma_start(b_sb[:], bt[:])

            ps = [
                pp.tile([3, NB], mybir.dt.float32, name=f"ps{i}", tag=f"ps{i}")
                for i in range(N_BANKS)
            ]

            def chunk_mms(m_tile, k1, a):
                lhsT = b_sb[:, k1 * 3:(k1 + 1) * 3]
                for nb in range(N_BANKS):
                    nc.tensor.matmul(
                        ps[nb][:, :],
                        lhsT,
                        m_tile[:, a * N_PER_CORE + nb * NB:
                               a * N_PER_CORE + (nb + 1) * NB],
                        start=(k1 == 0),
                        stop=(k1 == N_KCHUNKS - 1),
                    )

            # Main stream: big DMA tiles for all but the last qc chunks.
            for i in range(n_dmas - 1):
                m_sb = mpool.tile([KC, qc * N_PER_CORE], dt, name="m_sb")
                nc.sync.dma_start(m_sb[:], mt[i * KC:(i + 1) * KC, :])
                for a in range(qc):
                    chunk_mms(m_sb, i * qc + a, a)

            # Tail: last qc chunks arrive in qc/2-sized pieces so the PE's
            # final matmuls start as soon as each small piece lands instead
            # of waiting for a whole qc-chunk DMA.
            tq = 2
            for j in range(qc // tq):
                k_base = (n_dmas - 1) * qc + j * tq
                m_tl = mpool.tile(
                    [KC, tq * N_PER_CORE], dt, name="m_tl", tag="m_tl",
                    bufs=qc // tq,
                )
                nc.sync.dma_start(
                    m_tl[:],
                    mt[(n_dmas - 1) * KC:n_dmas * KC,
                       j * tq * N_PER_CORE:(j + 1) * tq * N_PER_CORE],
                )
                for a in range(tq):
                    chunk_mms(m_tl, k_base + a, a)

            # Epilogue: PSUM -> SBUF on two engines in parallel, then one DMA.
            o_sb = opool.tile([3, N_PER_CORE], mybir.dt.float32)
            for nb in range(N_BANKS):
                dst = o_sb[:, nb * NB:(nb + 1) * NB]
                if nb % 2 == 0:
                    nc.vector.tensor_copy(dst, ps[nb][:, :])
                else:
                    nc.scalar.copy(dst, ps[nb][:, :])
            nc.sync.dma_start(out[:], o_sb[:])

    nc.compile()
    return nc


def _get_program(dtype_name):
    if dtype_name not in _PROGRAM_CACHE:
        _PROGRAM_CACHE[dtype_name] = _build_program(dtype_name)
    return _PROGRAM_CACHE[dtype_name]


def _prepare_inputs(mapping, base_image, dtype_name):
    np_dt = _np_compute_dtype(dtype_name)
    # base [128,128,3] -> base_flat [P0, 3] -> [128 part, 128 kchunk * 3]
    # bt[p, k1*3 + c] = base_flat[k1*128 + p, c]
    base_flat = np.asarray(base_image, dtype=np.float32).reshape(P0, 3)
    bt = np.ascontiguousarray(
        base_flat.reshape(N_KCHUNKS, KC, 3).transpose(1, 0, 2).reshape(
            KC, N_KCHUNKS * 3
        )
    ).astype(np_dt)

    in_maps = []
    for c in range(N_CORES):
        shard = mapping[c * N_PER_CORE:(c + 1) * N_PER_CORE, :]  # [2048, P0] view
        mt_c = shard.T.astype(np_dt)  # [P0, 2048] K-major
        # tile-major: [tile i][partition p][(chunk a, n)] so each DMA tile is
        # one contiguous 32 KB read per partition (128 descriptors, not 1024)
        n_t = N_KCHUNKS // CHUNKS_PER_DMA
        mt_c = np.ascontiguousarray(
            mt_c.reshape(n_t, CHUNKS_PER_DMA, KC, N_PER_CORE).swapaxes(1, 2)
        ).reshape(n_t * KC, CHUNKS_PER_DMA * N_PER_CORE)
        in_maps.append({"mt": mt_c, "bt": bt})
    return in_maps


def _run(mapping, base_image, dtype_name, trace=False):
    nc = _get_program(dtype_name)
    in_maps = _prepare_inputs(mapping, base_image, dtype_name)
    res = run_bass_kernel_spmd(nc, in_maps, list(range(N_CORES)), trace=trace)
    mapped_flat = np.concatenate(
        [res.results[c]["out"].T for c in range(N_CORES)], axis=0
    )  # [P1, 3] f32
    mapped_image = mapped_flat.reshape(H1, W1, 3)
    return mapped_image, res


def kernel(mapping, base_image):
    mapping = np.asarray(mapping, dtype=np.float32)
    base_image = np.asarray(base_image, dtype=np.float32)
    mapped_image, _ = _run(mapping, base_image, COMPUTE_DTYPE)
    return (base_image, mapped_image)



# revision 3
# speedup vs baseline: 1.9628x; 1.9628x over previous
"""Trainium2 Bass kernel for BaseGenerator: mapped = mapping @ base_flat.

Strategy (8-core SPMD, pure data-parallel over output pixels):
  - mapping [P1=16384, P0=16384] f32 is row-sharded: core c owns output rows
    [c*2048, (c+1)*2048).  Host pre-transposes each shard to mt_c [P0, 2048]
    (K-major, cast to fp8 e4m3) so the contraction axis lands on SBUF
    partitions and the device streams the shard as a few large contiguous
    DMAs plus a fine-grained tail (so the final matmuls chase the stream).
  - base_flat [P0, 3] is replicated, rearranged host-side to [128, 128, 16]
    (3 fp8 values + 13 pad per K-chunk -> 16 B plane stride, as required by
    DoubleRow's weight access pattern).
  - Device: matmuls run in fp8 DoubleRow perf mode: each matmul contracts a
    *pair* of 128-row K-chunks ([128, 2, 512] moving AP, [128, 2, 3] weight
    AP) at 2 MACs/cell/cycle, halving PE streaming time vs fp16/bf16.
    64 pair-matmuls x 4 PSUM banks of [3, 512] f32 accumulate across the
    full K=16384.  Epilogue copies PSUM -> SBUF -> DRAM out [3, 2048] f32.
  - Host concatenates per-core outputs -> [16384, 3] -> [128, 128, 3].

The kernel is DMA-bound: 32 MiB/core (fp8) streams at the per-core HBM
limit (~360-400 GB/s); fp8 e4m3 quantization of both operands lands at
absmax rel err ~1.5e-3 vs the f32 reference (tolerance 2e-2).  The fp16
non-DoubleRow path is kept for A/B testing via COMPUTE_DTYPE.
"""

import sys

import numpy as np

try:
    import concourse.bacc as bacc
except ImportError:  # fresh env without PYTHONPATH: fall back to repo paths
    for _p in ("/opt/trn_rl_repo", "/opt/pypackages",
               "/root/.axon_site/_ro/trn_rl_repo",
               "/root/.axon_site/_ro/pypackages"):
        if _p not in sys.path:
            sys.path.append(_p)
    import concourse.bacc as bacc
import concourse.bass as bass
import concourse.mybir as mybir
import concourse.tile as tile
from concourse.bass_utils import run_bass_kernel_spmd

H0 = W0 = 128
H1 = W1 = 128
P0 = H0 * W0          # 16384 contraction length
P1 = H1 * W1          # 16384 output pixels
N_CORES = 8
N_PER_CORE = P1 // N_CORES   # 2048 output pixels per core
KC = 128              # K-chunk size (SBUF partitions)
N_KCHUNKS = P0 // KC  # 128
NB = 512              # matmul moving free dim (one PSUM bank of f32)
N_BANKS = N_PER_CORE // NB   # 4
BPAD = 16             # bytes per K-chunk of base weights (3 used + pad)

COMPUTE_DTYPE = "float8e4"   # fp8 DoubleRow; "float16" = old non-DR path
CHUNKS_PER_DMA = 8           # K-chunks fetched per dma_start (even!)
DMA_BUFS = 4                 # in-flight DMA tiles

_PROGRAM_CACHE = {}


def _np_compute_dtype(name):
    import ml_dtypes
    if name == "float32":
        return np.float32
    if name == "float16":
        return np.float16
    if name == "float8e4":
        return ml_dtypes.float8_e4m3fn
    return ml_dtypes.bfloat16


def _build_program(dtype_name):
    """Build + compile the SPMD Bass program (identical on all 8 cores)."""
    dt = getattr(mybir.dt, dtype_name)
    use_dr = dtype_name == "float8e4"
    nc = bacc.Bacc(
        "TRN2", target_bir_lowering=False, debug=False, num_devices=N_CORES
    )
    qc = CHUNKS_PER_DMA
    n_dmas = N_KCHUNKS // qc
    mt = nc.dram_tensor("mt", [n_dmas * KC, qc, N_PER_CORE], dt,
                        kind="ExternalInput")
    bt = nc.dram_tensor("bt", [KC, N_KCHUNKS, BPAD], dt, kind="ExternalInput")
    out = nc.dram_tensor(
        "out", [3, N_PER_CORE], mybir.dt.float32, kind="ExternalOutput"
    )

    # mt[(i*KC)+p, a, n] holds mapping^T K-chunk (i*qc + a) so one DMA tile
    # is a contiguous qc*N_PER_CORE-byte read per partition.
    with tile.TileContext(nc) as tc:
        with (
            tc.tile_pool(name="bpool", bufs=1) as bpool,
            tc.tile_pool(name="mpool", bufs=DMA_BUFS) as mpool,
            tc.tile_pool(name="psum", bufs=1, space=bass.MemorySpace.PSUM) as pp,
            tc.tile_pool(name="opool", bufs=1) as opool,
        ):
            b_sb = bpool.tile([KC, N_KCHUNKS, BPAD], dt)
            nc.sync.dma_start(b_sb[:], bt[:])

            ps = [
                pp.tile([3, NB], mybir.dt.float32, name=f"ps{i}", tag=f"ps{i}")
                for i in range(N_BANKS)
            ]

            def chunk_mms(m_tile, k1, a):
                """Matmuls for K-chunk(s) starting at global chunk k1 =
                local chunk a of m_tile.  DoubleRow consumes chunks (k1, k1+1)
                in one call; the plain path consumes just k1."""
                if use_dr:
                    lhsT = b_sb[:, k1:k1 + 2, 0:3]
                    for nb in range(N_BANKS):
                        nc.tensor.matmul(
                            ps[nb][:, :],
                            lhsT,
                            m_tile[:, a:a + 2, nb * NB:(nb + 1) * NB],
                            start=(k1 == 0),
                            stop=(k1 == N_KCHUNKS - 2),
                            perf_mode=mybir.MatmulPerfMode.DoubleRow,
                        )
                else:
                    lhsT = b_sb[:, k1:k1 + 1, 0:3]
                    for nb in range(N_BANKS):
                        nc.tensor.matmul(
                            ps[nb][:, :],
                            lhsT,
                            m_tile[:, a:a + 1, nb * NB:(nb + 1) * NB],
                            start=(k1 == 0),
                            stop=(k1 == N_KCHUNKS - 1),
                        )

            kstep = 2 if use_dr else 1

            # Main stream: big DMA tiles for all but the last qc chunks.
            for i in range(n_dmas - 1):
                m_sb = mpool.tile([KC, qc, N_PER_CORE], dt, name="m_sb")
                nc.sync.dma_start(m_sb[:], mt[i * KC:(i + 1) * KC])
                for a in range(0, qc, kstep):
                    chunk_mms(m_sb, i * qc + a, a)

            # Tail: last qc chunks arrive in tq-sized pieces so the PE's
            # final matmuls start as soon as each small piece lands instead
            # of waiting for a whole qc-chunk DMA.
            tq = 2
            for j in range(qc // tq):
                k_base = (n_dmas - 1) * qc + j * tq
                m_tl = mpool.tile(
                    [KC, tq, N_PER_CORE], dt, name="m_tl", tag="m_tl",
                    bufs=qc // tq,
                )
                nc.sync.dma_start(
                    m_tl[:],
                    mt[(n_dmas - 1) * KC:n_dmas * KC, j * tq:(j + 1) * tq],
                )
                for a in range(0, tq, kstep):
                    chunk_mms(m_tl, k_base + a, a)

            # Epilogue: PSUM -> SBUF on two engines in parallel, then one DMA.
            o_sb = opool.tile([3, N_PER_CORE], mybir.dt.float32)
            for nb in range(N_BANKS):
                dst = o_sb[:, nb * NB:(nb + 1) * NB]
                if nb % 2 == 0:
                    nc.vector.tensor_copy(dst, ps[nb][:, :])
                else:
                    nc.scalar.copy(dst, ps[nb][:, :])
            nc.sync.dma_start(out[:], o_sb[:])

    nc.compile()
    return nc


def _get_program(dtype_name):
    if dtype_name not in _PROGRAM_CACHE:
        _PROGRAM_CACHE[dtype_name] = _build_program(dtype_name)
    return _PROGRAM_CACHE[dtype_name]


def _prepare_inputs(mapping, base_image, dtype_name):
    np_dt = _np_compute_dtype(dtype_name)
    # base [128,128,3] -> base_flat [P0, 3] -> bt [128 part, 128 kchunk, 16]
    # bt[p, k1, c] = base_flat[k1*128 + p, c] for c < 3, 0-padded to 16.
    base_flat = np.asarray(base_image, dtype=np.float32).reshape(P0, 3)
    bt = np.zeros((KC, N_KCHUNKS, BPAD), dtype=np_dt)
    bt[:, :, 0:3] = base_flat.reshape(N_KCHUNKS, KC, 3).transpose(1, 0, 2)

    qc = CHUNKS_PER_DMA
    n_t = N_KCHUNKS // qc
    in_maps = []
    for c in range(N_CORES):
        shard = mapping[c * N_PER_CORE:(c + 1) * N_PER_CORE, :]  # [2048, P0] view
        mt_c = shard.T.astype(np_dt)  # [P0, 2048] K-major
        # tile-major: [tile i][partition p][chunk a][n] so each DMA tile is
        # one contiguous qc*2048 B read per partition.
        mt_c = np.ascontiguousarray(
            mt_c.reshape(n_t, qc, KC, N_PER_CORE).swapaxes(1, 2)
        ).reshape(n_t * KC, qc, N_PER_CORE)
        in_maps.append({"mt": mt_c, "bt": bt})
    return in_maps


def _run(mapping, base_image, dtype_name, trace=False):
    nc = _get_program(dtype_name)
    in_maps = _prepare_inputs(mapping, base_image, dtype_name)
    res = run_bass_kernel_spmd(nc, in_maps, list(range(N_CORES)), trace=trace)
    mapped_flat = np.concatenate(
        [res.results[c]["out"].T for c in range(N_CORES)], axis=0
    )  # [P1, 3] f32
    mapped_image = mapped_flat.reshape(H1, W1, 3)
    return mapped_image, res


def kernel(mapping, base_image):
    mapping = np.asarray(mapping, dtype=np.float32)
    base_image = np.asarray(base_image, dtype=np.float32)
    mapped_image, _ = _run(mapping, base_image, COMPUTE_DTYPE)
    return (base_image, mapped_image)
